# revision 24
# baseline (speedup 1.0000x reference)
"""Trainium2 Bass kernel for the DUAN conditioned-normalization problem (v2).

Contract: kernel(**inputs) takes FULL inputs (B=8 samples), shards one sample
per NeuronCore (8 cores), runs a single Bass/Tile kernel SPMD, and gathers the
full [8, 512, 8192] output.

Per-sample math (matches the jax reference):
  mu_c/var_c over L per channel; mu_l/var_l over (C,L);
  g = sigmoid(gw2 @ relu(gw1 @ c + gb1) + gb2); g_mix = mean_L(g)
  mu = g_mix*mu_c + (1-g_mix)*mu_l ; sigma likewise from sqrt(var+eps)
  gamma,beta = mw2 @ relu(mw1 @ mean_L(c) + mb1) + mb2
  y = (1+gamma)*(x-mu)/sigma + beta
  keep top-k channels by mean_L |y| (k=358), zero the rest.

v2: x staged host-side to fp16 and y written fp16 (upcast on host), halving
the dominant HBM traffic vs fp32 (in 16 MiB, out 8 MiB per core).  The gate
network runs in bf16 on the PE.  Top-k mask on-device via imp ranking.
Measured rel-err ~7e-4 with exact top-k mask agreement.
"""

import sys

sys.path.insert(0, "/opt/trn_rl_repo")

import numpy as np

B = 8
C = 512
L = 8192
H = 128
CG = 4           # channel groups of 128 partitions
SL = 1024        # gate-network super-tile width along L
NST = L // SL    # 8
XC = 4096        # x dma chunk along L
NXC = L // XC    # 2
LB = 512         # bn_stats chunk
NLB = L // LB    # 16
KEEP = max(1, int(C * 0.7))  # 358
EPS = 1e-5

_CACHE = {}


def _build_nc():
    import concourse.bacc as bacc
    import concourse.bass as bass
    import concourse.tile as tile
    from concourse import mybir

    f32 = mybir.dt.float32
    f16 = mybir.dt.float16
    bf16 = mybir.dt.bfloat16
    AF = mybir.ActivationFunctionType
    OP = mybir.AluOpType
    AX = mybir.AxisListType

    nc = bacc.Bacc("TRN2", target_bir_lowering=False, debug=False, num_devices=8)

    x_d = nc.declare_dram_parameter("x", [C, L], f16, isOutput=False)
    c_d = nc.declare_dram_parameter("c", [C, L], bf16, isOutput=False)
    gw1t_d = nc.declare_dram_parameter("gw1t", [128, CG, H], bf16, isOutput=False)
    gb1_d = nc.declare_dram_parameter("gb1c", [H, 1], f32, isOutput=False)
    gw2t_d = nc.declare_dram_parameter("gw2t", [H, C], bf16, isOutput=False)
    gb2_d = nc.declare_dram_parameter("gb2c", [H, CG], f32, isOutput=False)
    mw1t_d = nc.declare_dram_parameter("mw1t", [128, CG, H], bf16, isOutput=False)
    mb1_d = nc.declare_dram_parameter("mb1c", [H, 1], f32, isOutput=False)
    mw2t_d = nc.declare_dram_parameter("mw2t", [H, 2 * C], f32, isOutput=False)
    mb2_d = nc.declare_dram_parameter("mb2c", [H, 2 * CG], f32, isOutput=False)
    out_d = nc.declare_dram_parameter("out", [C, L], f16, isOutput=True)

    with tile.TileContext(nc) as tc:
        _emit(tc, bass, mybir, f32, f16, bf16, AF, OP, AX,
              x_d, c_d, gw1t_d, gb1_d, gw2t_d, gb2_d,
              mw1t_d, mb1_d, mw2t_d, mb2_d, out_d)

    nc.compile()
    return nc


def _emit(tc, bass, mybir, f32, f16, bf16, AF, OP, AX,
          x_d, c_d, gw1t_d, gb1_d, gw2t_d, gb2_d,
          mw1t_d, mb1_d, mw2t_d, mb2_d, out_d):
    from contextlib import ExitStack

    nc = tc.nc

    with ExitStack() as top:
        xpool = top.enter_context(tc.tile_pool(name="xbuf", bufs=1))
        wpool = top.enter_context(tc.tile_pool(name="wts", bufs=1))
        spool = top.enter_context(tc.tile_pool(name="stats", bufs=1))
        scpool = top.enter_context(tc.tile_pool(name="bigscr", bufs=2))
        dpool = top.enter_context(tc.tile_pool(name="dscr", bufs=1, space="DRAM"))
        imp_dr = dpool.tile([128, CG], f32, tag="impdr", name="impdr")

        # ---- weights / biases into SBUF ----
        w1_sb = wpool.tile([128, CG, H], bf16, tag="w1", name="w1")
        nc.sync.dma_start(out=w1_sb[:], in_=gw1t_d[:])
        m1_sb = wpool.tile([128, CG, H], bf16, tag="m1w", name="m1w")
        nc.sync.dma_start(out=m1_sb[:], in_=mw1t_d[:])
        w2_sb = wpool.tile([128, C], bf16, tag="w2", name="w2")
        nc.sync.dma_start(out=w2_sb[:], in_=gw2t_d[:])
        gb1_sb = wpool.tile([128, 1], f32, tag="gb1", name="gb1")
        nc.sync.dma_start(out=gb1_sb[:], in_=gb1_d[:])
        gb2_sb = wpool.tile([128, CG], f32, tag="gb2", name="gb2")
        nc.sync.dma_start(out=gb2_sb[:], in_=gb2_d[:])
        mb1_sb = wpool.tile([128, 1], f32, tag="mb1", name="mb1")
        nc.sync.dma_start(out=mb1_sb[:], in_=mb1_d[:])
        mb2_sb = wpool.tile([128, 2 * CG], f32, tag="mb2", name="mb2")
        nc.sync.dma_start(out=mb2_sb[:], in_=mb2_d[:])
        m2_sb = wpool.tile([128, 2 * C], f32, tag="m2w", name="m2w")
        nc.sync.dma_start(out=m2_sb[:], in_=mw2t_d[:])

        ones_sb = spool.tile([128, 128], f32, tag="ones", name="ones")
        nc.vector.memset(ones_sb[:], 1.0)

        # ---- persistent accumulators ----
        X_sb = [xpool.tile([128, L], f16, tag=f"X{g}", name=f"X{g}") for g in range(CG)]
        stats = [spool.tile([128, NLB, 6], f32, tag=f"bnst{g}", name=f"bnst{g}")
                 for g in range(CG)]
        muvar = spool.tile([128, CG, 2], f32, tag="muvar", name="muvar")
        csum = spool.tile([128, CG * NST], f32, tag="csum", name="csum")
        gacc = spool.tile([128, CG * NST], f32, tag="gacc", name="gacc")
        work = spool.tile([128, 16], f32, tag="work", name="work")
        mu4t = spool.tile([128, CG], f32, tag="mu4t", name="mu4t")
        sg4t = spool.tile([128, CG], f32, tag="sg4t", name="sg4t")
        scal = spool.tile([128, 8], f32, tag="scal", name="scal")
        bl_sb = spool.tile([128, 2], f32, tag="blb", name="blb")
        cs4h = spool.tile([128, CG], bf16, tag="cs4h", name="cs4h")
        hm_sb = spool.tile([128, 1], f32, tag="hm", name="hm")
        A4 = spool.tile([128, CG], f32, tag="A4", name="A4")
        B4 = spool.tile([128, CG], f32, tag="B4", name="B4")
        iacc = spool.tile([128, CG], f32, tag="iacc", name="iacc")
        imp4 = spool.tile([128, CG], f32, tag="imp4", name="imp4")
        T_sb = spool.tile([128, C], f32, tag="Tsb", name="Tsb")
        rank4 = spool.tile([128, CG], f32, tag="rank4", name="rank4")
        mask4 = spool.tile([128, CG], f32, tag="mask4", name="mask4")
        A4m = spool.tile([128, CG], f32, tag="A4m", name="A4m")
        B4m = spool.tile([128, CG], f32, tag="B4m", name="B4m")
        gscr512 = spool.tile([128, C], f32, tag="g512", name="g512")

        # =========================== phase 1 ===========================
        with ExitStack() as ph1:
            cpool = ph1.enter_context(tc.tile_pool(name="cbuf", bufs=3))
            hpool = ph1.enter_context(tc.tile_pool(name="hbuf", bufs=2))
            gspool = ph1.enter_context(tc.tile_pool(name="gscr", bufs=2))
            ps_h = ph1.enter_context(tc.tile_pool(name="psh", bufs=2, space="PSUM"))
            ps_g = ph1.enter_context(tc.tile_pool(name="psg", bufs=2, space="PSUM"))

            for st in range(NST):
                l0 = st * SL
                c_t = [cpool.tile([128, SL], bf16, tag=f"c{g}", name=f"c{g}")
                       for g in range(CG)]
                for g in range(CG):
                    nc.sync.dma_start(
                        out=c_t[g][:],
                        in_=c_d[g * 128:(g + 1) * 128, l0:l0 + SL],
                    )
                # one x chunk per super-tile: [128, XC] for group st//2
                xg, xj = divmod(st, NXC)
                nc.scalar.dma_start(
                    out=X_sb[xg][:, xj * XC:(xj + 1) * XC],
                    in_=x_d[xg * 128:(xg + 1) * 128, xj * XC:(xj + 1) * XC],
                )

                # gate layer 1: h = relu(gw1 @ c + gb1), contraction over C
                h_ps = ps_h.tile([128, SL], f32, tag="hps", name="hps")
                for g in range(CG):
                    for q in range(SL // 512):
                        hs = slice(q * 512, (q + 1) * 512)
                        nc.tensor.matmul(h_ps[:, hs], w1_sb[:, g, :], c_t[g][:, hs],
                                         start=(g == 0), stop=(g == CG - 1))
                h_sb = hpool.tile([128, SL], bf16, tag="hsb", name="hsb")
                nc.scalar.activation(out=h_sb[:], in_=h_ps[:], func=AF.Relu,
                                     bias=gb1_sb[:], scale=1.0)

                # c channel-sums: 3 groups on DVE, 1 on ACT
                for g in range(CG - 1):
                    nc.vector.reduce_sum(
                        out=csum[:, g * NST + st:g * NST + st + 1],
                        in_=c_t[g][:], axis=AX.X)
                cscr = gspool.tile([128, SL], bf16, tag="gscr", name="cscr")
                nc.scalar.activation(
                    out=cscr[:], in_=c_t[CG - 1][:], func=AF.Identity,
                    bias=0.0, scale=1.0,
                    accum_out=csum[:, (CG - 1) * NST + st:(CG - 1) * NST + st + 1])

                # gate layer 2 + sigmoid; mean over L via accum_out
                for g in range(CG):
                    g_ps = ps_g.tile([128, SL], f32, tag="gmps", name="g_ps")
                    for q in range(SL // 512):
                        hs = slice(q * 512, (q + 1) * 512)
                        nc.tensor.matmul(g_ps[:, hs], w2_sb[:, g * 128:(g + 1) * 128],
                                         h_sb[:, hs], start=True, stop=True)
                    g_scr = gspool.tile([128, SL], bf16, tag="gscr", name="g_scr")
                    nc.scalar.activation(out=g_scr[:], in_=g_ps[:], func=AF.Sigmoid,
                                         bias=gb2_sb[:, g:g + 1], scale=1.0,
                                         accum_out=gacc[:, g * NST + st:g * NST + st + 1])

                # x stats for the arrived chunk: bn_stats per 512-wide piece
                for jj in range(XC // LB):
                    j = xj * (XC // LB) + jj
                    nc.vector.bn_stats(out=stats[xg][:, j, :],
                                       in_=X_sb[xg][:, j * LB:(j + 1) * LB])
                if xj == NXC - 1:
                    nc.vector.bn_aggr(out=muvar[:, xg, :], in_=stats[xg][:])

        # =========================== finalize ===========================
        with ExitStack() as fin:
            ps_f = fin.enter_context(tc.tile_pool(name="psf", bufs=1, space="PSUM"))

            # ---- cond MLP first (independent of x stats; PE is idle) ----
            cs4 = work[:, 8:12]
            for g in range(CG):
                nc.vector.reduce_sum(out=cs4[:, g:g + 1],
                                     in_=csum[:, g * NST:(g + 1) * NST], axis=AX.X)
            nc.vector.tensor_copy(out=cs4h[:], in_=cs4)
            hm_ps = ps_f.tile([128, 1], f32, tag="hmps", name="hmps")
            for g in range(CG):
                nc.tensor.matmul(hm_ps[:], m1_sb[:, g, :], cs4h[:, g:g + 1],
                                 start=(g == 0), stop=(g == CG - 1))
            nc.scalar.activation(out=hm_sb[:], in_=hm_ps[:], func=AF.Relu,
                                 bias=mb1_sb[:], scale=1.0 / L)
            gb_ps = ps_f.tile([128, 2 * CG], f32, tag="gbps", name="gbps")
            for o in range(2 * CG):
                nc.tensor.matmul(gb_ps[:, o:o + 1],
                                 m2_sb[:, o * 128:(o + 1) * 128], hm_sb[:],
                                 start=True, stop=True)

            # ---- channel stats (bn_aggr already done per-group in phase 1) ----
            mu_c = muvar[:, :, 0]   # [128, CG] strided
            var_c = muvar[:, :, 1]

            # rowstats: cols 0:4 = mu_c, 4:8 = E[x^2] = var_c + mu_c^2
            nc.vector.tensor_copy(out=work[:, 0:4], in_=mu_c)
            nc.vector.tensor_tensor(out=work[:, 4:8], in0=mu_c, in1=mu_c, op=OP.mult)
            nc.vector.tensor_add(out=work[:, 4:8], in0=work[:, 4:8], in1=var_c)

            # cross-partition sums -> [1, 8]
            colsum = ps_f.tile([128, 8], f32, tag="colsum", name="colsum")
            nc.tensor.matmul(colsum[0:1, :], ones_sb[:, 0:1], work[:, 0:8],
                             start=True, stop=True)

            # partition-0 scalar math: mu_l, var_l, sigma_l
            nc.vector.reduce_sum(out=scal[0:1, 0:1], in_=colsum[0:1, 0:4], axis=AX.X)
            nc.vector.tensor_scalar(out=scal[0:1, 0:1], in0=scal[0:1, 0:1],
                                    scalar1=1.0 / C, scalar2=None, op0=OP.mult)
            nc.vector.reduce_sum(out=scal[0:1, 2:3], in_=colsum[0:1, 4:8], axis=AX.X)
            nc.vector.tensor_scalar(out=scal[0:1, 2:3], in0=scal[0:1, 2:3],
                                    scalar1=1.0 / C, scalar2=None, op0=OP.mult)
            nc.vector.tensor_tensor(out=scal[0:1, 3:4], in0=scal[0:1, 0:1],
                                    in1=scal[0:1, 0:1], op=OP.mult)
            nc.vector.tensor_tensor(out=scal[0:1, 1:2], in0=scal[0:1, 2:3],
                                    in1=scal[0:1, 3:4], op=OP.subtract)
            # sigma_l = sqrt(var_l + eps), one Newton polish
            nc.vector.tensor_scalar(out=scal[0:1, 4:5], in0=scal[0:1, 1:2],
                                    scalar1=EPS, scalar2=None, op0=OP.add)
            nc.scalar.activation(out=scal[0:1, 1:2], in_=scal[0:1, 4:5],
                                 func=AF.Sqrt, bias=0.0, scale=1.0)
            nc.vector.reciprocal(out=scal[0:1, 5:6], in_=scal[0:1, 1:2])
            nc.vector.tensor_tensor(out=scal[0:1, 5:6], in0=scal[0:1, 4:5],
                                    in1=scal[0:1, 5:6], op=OP.mult)
            nc.vector.tensor_add(out=scal[0:1, 1:2], in0=scal[0:1, 1:2],
                                 in1=scal[0:1, 5:6])
            nc.vector.tensor_scalar(out=scal[0:1, 1:2], in0=scal[0:1, 1:2],
                                    scalar1=0.5, scalar2=None, op0=OP.mult)

            # broadcast (mu_l, sigma_l) to all partitions
            bl_ps = ps_f.tile([128, 2], f32, tag="blps", name="blps")
            nc.tensor.matmul(bl_ps[:], ones_sb[0:1, :], scal[0:1, 0:2],
                             start=True, stop=True)
            nc.vector.tensor_copy(out=bl_sb[:], in_=bl_ps[:])
            mu_l = bl_sb[:, 0:1]
            sig_l = bl_sb[:, 1:2]

            # sigma_c = sqrt(var_c + eps) + Newton polish
            vpe4 = work[:, 12:16]
            nc.vector.tensor_scalar(out=vpe4, in0=var_c, scalar1=EPS,
                                    scalar2=None, op0=OP.add)
            sig4 = work[:, 8:12]
            nc.scalar.activation(out=sig4, in_=vpe4, func=AF.Sqrt,
                                 bias=0.0, scale=1.0)
            t4s = work[:, 4:8]
            nc.vector.reciprocal(out=t4s, in_=sig4)
            nc.vector.tensor_tensor(out=t4s, in0=vpe4, in1=t4s, op=OP.mult)
            nc.vector.tensor_add(out=sig4, in0=sig4, in1=t4s)
            nc.vector.tensor_scalar(out=sig4, in0=sig4, scalar1=0.5,
                                    scalar2=None, op0=OP.mult)

            # g_mix = sum(gacc) / L
            gm4 = work[:, 12:16]
            for g in range(CG):
                nc.vector.reduce_sum(out=gm4[:, g:g + 1],
                                     in_=gacc[:, g * NST:(g + 1) * NST], axis=AX.X)
            nc.vector.tensor_scalar(out=gm4, in0=gm4, scalar1=1.0 / L,
                                    scalar2=None, op0=OP.mult)

            # mu = mu_l + g_mix*(mu_c - mu_l); sigma likewise
            nc.vector.tensor_scalar(out=mu4t[:], in0=work[:, 0:4], scalar1=mu_l,
                                    scalar2=None, op0=OP.subtract)
            nc.vector.tensor_tensor(out=mu4t[:], in0=mu4t[:], in1=gm4, op=OP.mult)
            nc.vector.tensor_scalar(out=mu4t[:], in0=mu4t[:], scalar1=mu_l,
                                    scalar2=None, op0=OP.add)
            nc.vector.tensor_scalar(out=sg4t[:], in0=sig4, scalar1=sig_l,
                                    scalar2=None, op0=OP.subtract)
            nc.vector.tensor_tensor(out=sg4t[:], in0=sg4t[:], in1=gm4, op=OP.mult)
            nc.vector.tensor_scalar(out=sg4t[:], in0=sg4t[:], scalar1=sig_l,
                                    scalar2=None, op0=OP.add)

            # A = (1+gamma)/sigma ; B = beta - mu*A
            inv4 = work[:, 0:4]
            nc.vector.reciprocal(out=inv4, in_=sg4t[:])
            gam4 = work[:, 4:8]
            nc.vector.tensor_add(out=gam4, in0=gb_ps[:, 0:CG], in1=mb2_sb[:, 0:CG])
            nc.vector.tensor_scalar(out=gam4, in0=gam4, scalar1=1.0,
                                    scalar2=None, op0=OP.add)
            bet4 = work[:, 8:12]
            nc.vector.tensor_add(out=bet4, in0=gb_ps[:, CG:2 * CG],
                                 in1=mb2_sb[:, CG:2 * CG])
            nc.vector.tensor_tensor(out=A4[:], in0=gam4, in1=inv4, op=OP.mult)
            muA = work[:, 12:16]
            nc.vector.tensor_tensor(out=muA, in0=mu4t[:], in1=A4[:], op=OP.mult)
            nc.vector.tensor_tensor(out=B4[:], in0=bet4, in1=muA, op=OP.subtract)

            # ---- imp_g = sum_L |A x + B|: ACT for groups 0-2, DVE for 3 ----
            for g in range(CG):
                if g < 3:
                    iscr = scpool.tile([128, L], bf16, tag="bigscr", name="iscr")
                    nc.scalar.activation(out=iscr[:], in_=X_sb[g][:], func=AF.Abs,
                                         bias=B4[:, g:g + 1], scale=A4[:, g:g + 1],
                                         accum_out=iacc[:, g:g + 1])
                else:
                    dscr = scpool.tile([128, L], f32, tag="bigscr32", name="dscr")
                    nc.vector.tensor_scalar(out=dscr[:], in0=X_sb[g][:],
                                            scalar1=A4[:, g:g + 1],
                                            scalar2=B4[:, g:g + 1],
                                            op0=OP.mult, op1=OP.add)
                    nc.vector.tensor_reduce(out=iacc[:, g:g + 1], in_=dscr[:],
                                            axis=AX.X, op=OP.add,
                                            apply_absolute_value=True)
            nc.vector.tensor_copy(out=imp4[:], in_=iacc[:])

            # ---- top-k mask: rank[i] = #{j: imp[j] > imp[i]} ----
            nc.scalar.dma_start(out=imp_dr[:], in_=imp4[:])
            imp_flat = imp_dr[:]
            bcast = bass.AP(tensor=imp_flat.tensor, offset=imp_flat.offset,
                            ap=[[0, 128], [1, C]])
            nc.scalar.dma_start(out=T_sb[:], in_=bcast)
            for g in range(CG):
                nc.vector.tensor_scalar(out=gscr512[:], in0=T_sb[:],
                                        scalar1=imp4[:, g:g + 1], scalar2=0.0,
                                        op0=OP.is_gt, op1=OP.add,
                                        accum_out=rank4[:, g:g + 1])
            nc.vector.tensor_scalar(out=mask4[:], in0=rank4[:], scalar1=float(KEEP),
                                    scalar2=None, op0=OP.is_lt)
            nc.vector.tensor_tensor(out=A4m[:], in0=A4[:], in1=mask4[:], op=OP.mult)
            nc.vector.tensor_tensor(out=B4m[:], in0=B4[:], in1=mask4[:], op=OP.mult)

        # =========================== phase 2: write y ===========================
        with ExitStack() as ph2:
            ypool = ph2.enter_context(tc.tile_pool(name="ybuf", bufs=2))
            for g in range(CG):
                y_t = ypool.tile([128, L], f16, tag="yt", name="yt")
                nc.vector.tensor_scalar(out=y_t[:], in0=X_sb[g][:],
                                        scalar1=A4m[:, g:g + 1],
                                        scalar2=B4m[:, g:g + 1],
                                        op0=OP.mult, op1=OP.add)
                eng = nc.sync if g % 2 == 0 else nc.scalar
                eng.dma_start(out=out_d[g * 128:(g + 1) * 128, :], in_=y_t[:])


def _get_nc():
    if "nc" not in _CACHE:
        _CACHE["nc"] = _build_nc()
    return _CACHE["nc"]


def _host_weight_maps(gw1, gb1, gw2, gb2, mw1, mb1, mw2, mb2):
    import ml_dtypes
    f = np.float32
    bf = ml_dtypes.bfloat16
    return {
        "gw1t": np.ascontiguousarray(np.asarray(gw1, f).T.reshape(CG, 128, H).transpose(1, 0, 2).astype(bf)),
        "gb1c": np.ascontiguousarray(np.asarray(gb1, f).reshape(H, 1)),
        "gw2t": np.ascontiguousarray(np.asarray(gw2, f).T.astype(bf)),   # [H,C]
        "gb2c": np.ascontiguousarray(np.asarray(gb2, f).reshape(CG, 128).T),
        "mw1t": np.ascontiguousarray(np.asarray(mw1, f).T.reshape(CG, 128, H).transpose(1, 0, 2).astype(bf)),
        "mb1c": np.ascontiguousarray(np.asarray(mb1, f).reshape(H, 1)),
        "mw2t": np.ascontiguousarray(np.asarray(mw2, f).T),              # [H,2C]
        "mb2c": np.ascontiguousarray(np.asarray(mb2, f).reshape(2 * CG, 128).T),
    }


def _run(inputs, trace=False):
    import ml_dtypes
    from concourse.bass_utils import run_bass_kernel_spmd

    nc = _get_nc()
    x = np.asarray(inputs["x"], np.float32).astype(np.float16)
    c = np.asarray(inputs["c"], np.float32).astype(ml_dtypes.bfloat16)
    wmap = _host_weight_maps(
        inputs["gw1"], inputs["gb1"], inputs["gw2"], inputs["gb2"],
        inputs["mw1"], inputs["mb1"], inputs["mw2"], inputs["mb2"])
    in_maps = [
        dict(wmap, x=np.ascontiguousarray(x[b]), c=np.ascontiguousarray(c[b]))
        for b in range(B)
    ]
    res = run_bass_kernel_spmd(nc, in_maps, core_ids=list(range(B)), trace=trace)
    out = np.stack([res.results[b]["out"] for b in range(B)], axis=0).astype(np.float32)
    return out, res


def kernel(**inputs):
    out, _ = _run(inputs, trace=False)
    return out


# revision 25
# speedup vs baseline: 1.3861x; 1.3861x over previous
"""Trainium2 Bass kernel for the DUAN conditioned-normalization problem (v2).

Contract: kernel(**inputs) takes FULL inputs (B=8 samples), shards one sample
per NeuronCore (8 cores), runs a single Bass/Tile kernel SPMD, and gathers the
full [8, 512, 8192] output.

Per-sample math (matches the jax reference):
  mu_c/var_c over L per channel; mu_l/var_l over (C,L);
  g = sigmoid(gw2 @ relu(gw1 @ c + gb1) + gb2); g_mix = mean_L(g)
  mu = g_mix*mu_c + (1-g_mix)*mu_l ; sigma likewise from sqrt(var+eps)
  gamma,beta = mw2 @ relu(mw1 @ mean_L(c) + mb1) + mb2
  y = (1+gamma)*(x-mu)/sigma + beta
  keep top-k channels by mean_L |y| (k=358), zero the rest.

v2: x staged host-side to fp16 and y written fp16 (upcast on host), halving
the dominant HBM traffic vs fp32 (in 16 MiB, out 8 MiB per core).  The gate
network runs in bf16 on the PE.  Top-k mask on-device via imp ranking.
Measured rel-err ~7e-4 with exact top-k mask agreement.
"""

import sys

sys.path.insert(0, "/opt/trn_rl_repo")

import numpy as np

B = 8
C = 512
L = 8192
H = 128
CG = 4           # channel groups of 128 partitions
SL = 1024        # gate-network super-tile width along L
NST = L // SL    # 8
XC = 4096        # x dma chunk along L
NXC = L // XC    # 2
LB = 512         # bn_stats chunk
NLB = L // LB    # 16
KEEP = max(1, int(C * 0.7))  # 358
EPS = 1e-5

_CACHE = {}


def _build_nc():
    import concourse.bacc as bacc
    import concourse.bass as bass
    import concourse.tile as tile
    from concourse import mybir

    f32 = mybir.dt.float32
    f16 = mybir.dt.float16
    bf16 = mybir.dt.bfloat16
    AF = mybir.ActivationFunctionType
    OP = mybir.AluOpType
    AX = mybir.AxisListType

    nc = bacc.Bacc("TRN2", target_bir_lowering=False, debug=False, num_devices=8)

    x_d = nc.declare_dram_parameter("x", [C, L], f16, isOutput=False)
    c_d = nc.declare_dram_parameter("c", [C, L], bf16, isOutput=False)
    gw1t_d = nc.declare_dram_parameter("gw1t", [128, CG, H], bf16, isOutput=False)
    gb1_d = nc.declare_dram_parameter("gb1c", [H, 1], f32, isOutput=False)
    gw2t_d = nc.declare_dram_parameter("gw2t", [H, C], bf16, isOutput=False)
    gb2_d = nc.declare_dram_parameter("gb2c", [H, CG], f32, isOutput=False)
    mw1t_d = nc.declare_dram_parameter("mw1t", [128, CG, H], bf16, isOutput=False)
    mb1_d = nc.declare_dram_parameter("mb1c", [H, 1], f32, isOutput=False)
    mw2t_d = nc.declare_dram_parameter("mw2t", [H, 2 * C], f32, isOutput=False)
    mb2_d = nc.declare_dram_parameter("mb2c", [H, 2 * CG], f32, isOutput=False)
    out_d = nc.declare_dram_parameter("out", [C, L], f16, isOutput=True)

    with tile.TileContext(nc) as tc:
        _emit(tc, bass, mybir, f32, f16, bf16, AF, OP, AX,
              x_d, c_d, gw1t_d, gb1_d, gw2t_d, gb2_d,
              mw1t_d, mb1_d, mw2t_d, mb2_d, out_d)

    nc.compile()
    return nc


def _emit(tc, bass, mybir, f32, f16, bf16, AF, OP, AX,
          x_d, c_d, gw1t_d, gb1_d, gw2t_d, gb2_d,
          mw1t_d, mb1_d, mw2t_d, mb2_d, out_d):
    from contextlib import ExitStack

    nc = tc.nc

    with ExitStack() as top:
        xpool = top.enter_context(tc.tile_pool(name="xbuf", bufs=1))
        wpool = top.enter_context(tc.tile_pool(name="wts", bufs=1))
        spool = top.enter_context(tc.tile_pool(name="stats", bufs=1))
        scpool = top.enter_context(tc.tile_pool(name="bigscr", bufs=2))
        dpool = top.enter_context(tc.tile_pool(name="dscr", bufs=1, space="DRAM"))
        imp_dr = dpool.tile([128, CG], f32, tag="impdr", name="impdr")

        # ---- weights / biases into SBUF ----
        w1_sb = wpool.tile([128, CG, H], bf16, tag="w1", name="w1")
        nc.sync.dma_start(out=w1_sb[:], in_=gw1t_d[:])
        m1_sb = wpool.tile([128, CG, H], bf16, tag="m1w", name="m1w")
        nc.sync.dma_start(out=m1_sb[:], in_=mw1t_d[:])
        w2_sb = wpool.tile([128, C], bf16, tag="w2", name="w2")
        nc.sync.dma_start(out=w2_sb[:], in_=gw2t_d[:])
        gb1_sb = wpool.tile([128, 1], f32, tag="gb1", name="gb1")
        nc.sync.dma_start(out=gb1_sb[:], in_=gb1_d[:])
        gb2_sb = wpool.tile([128, CG], f32, tag="gb2", name="gb2")
        nc.sync.dma_start(out=gb2_sb[:], in_=gb2_d[:])
        mb1_sb = wpool.tile([128, 1], f32, tag="mb1", name="mb1")
        nc.sync.dma_start(out=mb1_sb[:], in_=mb1_d[:])
        mb2_sb = wpool.tile([128, 2 * CG], f32, tag="mb2", name="mb2")
        nc.sync.dma_start(out=mb2_sb[:], in_=mb2_d[:])
        m2_sb = wpool.tile([128, 2 * C], f32, tag="m2w", name="m2w")
        nc.sync.dma_start(out=m2_sb[:], in_=mw2t_d[:])

        ones_sb = spool.tile([128, 128], f32, tag="ones", name="ones")
        nc.vector.memset(ones_sb[:], 1.0)

        # ---- persistent accumulators ----
        X_sb = [xpool.tile([128, L], f16, tag=f"X{g}", name=f"X{g}") for g in range(CG)]
        stats = [spool.tile([128, NLB, 6], f32, tag=f"bnst{g}", name=f"bnst{g}")
                 for g in range(CG)]
        muvar = spool.tile([128, CG, 2], f32, tag="muvar", name="muvar")
        csum = spool.tile([128, CG * NST], f32, tag="csum", name="csum")
        gacc = spool.tile([128, CG * NST], f32, tag="gacc", name="gacc")
        work = spool.tile([128, 16], f32, tag="work", name="work")
        mu4t = spool.tile([128, CG], f32, tag="mu4t", name="mu4t")
        sg4t = spool.tile([128, CG], f32, tag="sg4t", name="sg4t")
        scal = spool.tile([128, 8], f32, tag="scal", name="scal")
        bl_sb = spool.tile([128, 2], f32, tag="blb", name="blb")
        cs4h = spool.tile([128, CG], bf16, tag="cs4h", name="cs4h")
        hm_sb = spool.tile([128, 1], f32, tag="hm", name="hm")
        A4 = spool.tile([128, CG], f32, tag="A4", name="A4")
        B4 = spool.tile([128, CG], f32, tag="B4", name="B4")
        iacc = spool.tile([128, CG], f32, tag="iacc", name="iacc")
        imp4 = spool.tile([128, CG], f32, tag="imp4", name="imp4")
        T_sb = spool.tile([128, C], f32, tag="Tsb", name="Tsb")
        rank4 = spool.tile([128, CG], f32, tag="rank4", name="rank4")
        mask4 = spool.tile([128, CG], f32, tag="mask4", name="mask4")
        A4m = spool.tile([128, CG], f32, tag="A4m", name="A4m")
        B4m = spool.tile([128, CG], f32, tag="B4m", name="B4m")
        gscr512 = spool.tile([128, C], f32, tag="g512", name="g512")

        # =========================== phase 1 ===========================
        with ExitStack() as ph1:
            cpool = ph1.enter_context(tc.tile_pool(name="cbuf", bufs=3))
            hpool = ph1.enter_context(tc.tile_pool(name="hbuf", bufs=2))
            gspool = ph1.enter_context(tc.tile_pool(name="gscr", bufs=2))
            ps_h = ph1.enter_context(tc.tile_pool(name="psh", bufs=2, space="PSUM"))
            ps_g = ph1.enter_context(tc.tile_pool(name="psg", bufs=2, space="PSUM"))

            for st in range(NST):
                l0 = st * SL
                c_t = [cpool.tile([128, SL], bf16, tag=f"c{g}", name=f"c{g}")
                       for g in range(CG)]
                for g in range(CG):
                    nc.sync.dma_start(
                        out=c_t[g][:],
                        in_=c_d[g * 128:(g + 1) * 128, l0:l0 + SL],
                    )
                # one x chunk per super-tile: [128, XC] for group st//2
                xg, xj = divmod(st, NXC)
                nc.sync.dma_start(
                    out=X_sb[xg][:, xj * XC:(xj + 1) * XC],
                    in_=x_d[xg * 128:(xg + 1) * 128, xj * XC:(xj + 1) * XC],
                )

                # gate layer 1: h = relu(gw1 @ c + gb1), contraction over C
                h_ps = ps_h.tile([128, SL], f32, tag="hps", name="hps")
                for g in range(CG):
                    for q in range(SL // 512):
                        hs = slice(q * 512, (q + 1) * 512)
                        nc.tensor.matmul(h_ps[:, hs], w1_sb[:, g, :], c_t[g][:, hs],
                                         start=(g == 0), stop=(g == CG - 1))
                h_sb = hpool.tile([128, SL], bf16, tag="hsb", name="hsb")
                nc.scalar.activation(out=h_sb[:], in_=h_ps[:], func=AF.Relu,
                                     bias=gb1_sb[:], scale=1.0)

                # c channel-sums: 3 groups on DVE, 1 on ACT
                for g in range(CG - 1):
                    nc.vector.reduce_sum(
                        out=csum[:, g * NST + st:g * NST + st + 1],
                        in_=c_t[g][:], axis=AX.X)
                cscr = gspool.tile([128, SL], bf16, tag="gscr", name="cscr")
                nc.scalar.activation(
                    out=cscr[:], in_=c_t[CG - 1][:], func=AF.Identity,
                    bias=0.0, scale=1.0,
                    accum_out=csum[:, (CG - 1) * NST + st:(CG - 1) * NST + st + 1])

                # gate layer 2 + sigmoid; mean over L via accum_out
                for g in range(CG):
                    g_ps = ps_g.tile([128, SL], f32, tag="gmps", name="g_ps")
                    for q in range(SL // 512):
                        hs = slice(q * 512, (q + 1) * 512)
                        nc.tensor.matmul(g_ps[:, hs], w2_sb[:, g * 128:(g + 1) * 128],
                                         h_sb[:, hs], start=True, stop=True)
                    g_scr = gspool.tile([128, SL], bf16, tag="gscr", name="g_scr")
                    nc.scalar.activation(out=g_scr[:], in_=g_ps[:], func=AF.Sigmoid,
                                         bias=gb2_sb[:, g:g + 1], scale=1.0,
                                         accum_out=gacc[:, g * NST + st:g * NST + st + 1])

                # x stats for the arrived chunk: bn_stats per 512-wide piece
                for jj in range(XC // LB):
                    j = xj * (XC // LB) + jj
                    nc.vector.bn_stats(out=stats[xg][:, j, :],
                                       in_=X_sb[xg][:, j * LB:(j + 1) * LB])
                if xj == NXC - 1:
                    nc.vector.bn_aggr(out=muvar[:, xg, :], in_=stats[xg][:])

        # =========================== finalize ===========================
        with ExitStack() as fin:
            ps_f = fin.enter_context(tc.tile_pool(name="psf", bufs=1, space="PSUM"))

            # ---- cond MLP first (independent of x stats; PE is idle) ----
            cs4 = work[:, 8:12]
            for g in range(CG):
                nc.vector.reduce_sum(out=cs4[:, g:g + 1],
                                     in_=csum[:, g * NST:(g + 1) * NST], axis=AX.X)
            nc.vector.tensor_copy(out=cs4h[:], in_=cs4)
            hm_ps = ps_f.tile([128, 1], f32, tag="hmps", name="hmps")
            for g in range(CG):
                nc.tensor.matmul(hm_ps[:], m1_sb[:, g, :], cs4h[:, g:g + 1],
                                 start=(g == 0), stop=(g == CG - 1))
            nc.scalar.activation(out=hm_sb[:], in_=hm_ps[:], func=AF.Relu,
                                 bias=mb1_sb[:], scale=1.0 / L)
            gb_ps = ps_f.tile([128, 2 * CG], f32, tag="gbps", name="gbps")
            for o in range(2 * CG):
                nc.tensor.matmul(gb_ps[:, o:o + 1],
                                 m2_sb[:, o * 128:(o + 1) * 128], hm_sb[:],
                                 start=True, stop=True)

            # ---- channel stats (bn_aggr already done per-group in phase 1) ----
            mu_c = muvar[:, :, 0]   # [128, CG] strided
            var_c = muvar[:, :, 1]

            # rowstats: cols 0:4 = mu_c, 4:8 = E[x^2] = var_c + mu_c^2
            nc.vector.tensor_copy(out=work[:, 0:4], in_=mu_c)
            nc.vector.tensor_tensor(out=work[:, 4:8], in0=mu_c, in1=mu_c, op=OP.mult)
            nc.vector.tensor_add(out=work[:, 4:8], in0=work[:, 4:8], in1=var_c)

            # cross-partition sums -> [1, 8]
            colsum = ps_f.tile([128, 8], f32, tag="colsum", name="colsum")
            nc.tensor.matmul(colsum[0:1, :], ones_sb[:, 0:1], work[:, 0:8],
                             start=True, stop=True)

            # partition-0 scalar math: mu_l, var_l, sigma_l
            nc.vector.reduce_sum(out=scal[0:1, 0:1], in_=colsum[0:1, 0:4], axis=AX.X)
            nc.vector.tensor_scalar(out=scal[0:1, 0:1], in0=scal[0:1, 0:1],
                                    scalar1=1.0 / C, scalar2=None, op0=OP.mult)
            nc.vector.reduce_sum(out=scal[0:1, 2:3], in_=colsum[0:1, 4:8], axis=AX.X)
            nc.vector.tensor_scalar(out=scal[0:1, 2:3], in0=scal[0:1, 2:3],
                                    scalar1=1.0 / C, scalar2=None, op0=OP.mult)
            nc.vector.tensor_tensor(out=scal[0:1, 3:4], in0=scal[0:1, 0:1],
                                    in1=scal[0:1, 0:1], op=OP.mult)
            nc.vector.tensor_tensor(out=scal[0:1, 1:2], in0=scal[0:1, 2:3],
                                    in1=scal[0:1, 3:4], op=OP.subtract)
            # sigma_l = sqrt(var_l + eps), one Newton polish
            nc.vector.tensor_scalar(out=scal[0:1, 4:5], in0=scal[0:1, 1:2],
                                    scalar1=EPS, scalar2=None, op0=OP.add)
            nc.scalar.activation(out=scal[0:1, 1:2], in_=scal[0:1, 4:5],
                                 func=AF.Sqrt, bias=0.0, scale=1.0)
            nc.vector.reciprocal(out=scal[0:1, 5:6], in_=scal[0:1, 1:2])
            nc.vector.tensor_tensor(out=scal[0:1, 5:6], in0=scal[0:1, 4:5],
                                    in1=scal[0:1, 5:6], op=OP.mult)
            nc.vector.tensor_add(out=scal[0:1, 1:2], in0=scal[0:1, 1:2],
                                 in1=scal[0:1, 5:6])
            nc.vector.tensor_scalar(out=scal[0:1, 1:2], in0=scal[0:1, 1:2],
                                    scalar1=0.5, scalar2=None, op0=OP.mult)

            # broadcast (mu_l, sigma_l) to all partitions
            bl_ps = ps_f.tile([128, 2], f32, tag="blps", name="blps")
            nc.tensor.matmul(bl_ps[:], ones_sb[0:1, :], scal[0:1, 0:2],
                             start=True, stop=True)
            nc.vector.tensor_copy(out=bl_sb[:], in_=bl_ps[:])
            mu_l = bl_sb[:, 0:1]
            sig_l = bl_sb[:, 1:2]

            # sigma_c = sqrt(var_c + eps) + Newton polish
            vpe4 = work[:, 12:16]
            nc.vector.tensor_scalar(out=vpe4, in0=var_c, scalar1=EPS,
                                    scalar2=None, op0=OP.add)
            sig4 = work[:, 8:12]
            nc.scalar.activation(out=sig4, in_=vpe4, func=AF.Sqrt,
                                 bias=0.0, scale=1.0)
            t4s = work[:, 4:8]
            nc.vector.reciprocal(out=t4s, in_=sig4)
            nc.vector.tensor_tensor(out=t4s, in0=vpe4, in1=t4s, op=OP.mult)
            nc.vector.tensor_add(out=sig4, in0=sig4, in1=t4s)
            nc.vector.tensor_scalar(out=sig4, in0=sig4, scalar1=0.5,
                                    scalar2=None, op0=OP.mult)

            # g_mix = sum(gacc) / L
            gm4 = work[:, 12:16]
            for g in range(CG):
                nc.vector.reduce_sum(out=gm4[:, g:g + 1],
                                     in_=gacc[:, g * NST:(g + 1) * NST], axis=AX.X)
            nc.vector.tensor_scalar(out=gm4, in0=gm4, scalar1=1.0 / L,
                                    scalar2=None, op0=OP.mult)

            # mu = mu_l + g_mix*(mu_c - mu_l); sigma likewise
            nc.vector.tensor_scalar(out=mu4t[:], in0=work[:, 0:4], scalar1=mu_l,
                                    scalar2=None, op0=OP.subtract)
            nc.vector.tensor_tensor(out=mu4t[:], in0=mu4t[:], in1=gm4, op=OP.mult)
            nc.vector.tensor_scalar(out=mu4t[:], in0=mu4t[:], scalar1=mu_l,
                                    scalar2=None, op0=OP.add)
            nc.vector.tensor_scalar(out=sg4t[:], in0=sig4, scalar1=sig_l,
                                    scalar2=None, op0=OP.subtract)
            nc.vector.tensor_tensor(out=sg4t[:], in0=sg4t[:], in1=gm4, op=OP.mult)
            nc.vector.tensor_scalar(out=sg4t[:], in0=sg4t[:], scalar1=sig_l,
                                    scalar2=None, op0=OP.add)

            # A = (1+gamma)/sigma ; B = beta - mu*A
            inv4 = work[:, 0:4]
            nc.vector.reciprocal(out=inv4, in_=sg4t[:])
            gam4 = work[:, 4:8]
            nc.vector.tensor_add(out=gam4, in0=gb_ps[:, 0:CG], in1=mb2_sb[:, 0:CG])
            nc.vector.tensor_scalar(out=gam4, in0=gam4, scalar1=1.0,
                                    scalar2=None, op0=OP.add)
            bet4 = work[:, 8:12]
            nc.vector.tensor_add(out=bet4, in0=gb_ps[:, CG:2 * CG],
                                 in1=mb2_sb[:, CG:2 * CG])
            nc.vector.tensor_tensor(out=A4[:], in0=gam4, in1=inv4, op=OP.mult)
            muA = work[:, 12:16]
            nc.vector.tensor_tensor(out=muA, in0=mu4t[:], in1=A4[:], op=OP.mult)
            nc.vector.tensor_tensor(out=B4[:], in0=bet4, in1=muA, op=OP.subtract)

            # ---- imp_g = sum_L |A x + B|: ACT for groups 0-2, DVE for 3 ----
            for g in range(CG):
                if g < 3:
                    iscr = scpool.tile([128, L], bf16, tag="bigscr", name="iscr")
                    nc.scalar.activation(out=iscr[:], in_=X_sb[g][:], func=AF.Abs,
                                         bias=B4[:, g:g + 1], scale=A4[:, g:g + 1],
                                         accum_out=iacc[:, g:g + 1])
                else:
                    dscr = scpool.tile([128, L], f32, tag="bigscr32", name="dscr")
                    nc.vector.tensor_scalar(out=dscr[:], in0=X_sb[g][:],
                                            scalar1=A4[:, g:g + 1],
                                            scalar2=B4[:, g:g + 1],
                                            op0=OP.mult, op1=OP.add)
                    nc.vector.tensor_reduce(out=iacc[:, g:g + 1], in_=dscr[:],
                                            axis=AX.X, op=OP.add,
                                            apply_absolute_value=True)
            nc.vector.tensor_copy(out=imp4[:], in_=iacc[:])

            # ---- top-k mask: rank[i] = #{j: imp[j] > imp[i]} ----
            nc.scalar.dma_start(out=imp_dr[:], in_=imp4[:])
            imp_flat = imp_dr[:]
            bcast = bass.AP(tensor=imp_flat.tensor, offset=imp_flat.offset,
                            ap=[[0, 128], [1, C]])
            nc.scalar.dma_start(out=T_sb[:], in_=bcast)
            for g in range(CG):
                nc.vector.tensor_scalar(out=gscr512[:], in0=T_sb[:],
                                        scalar1=imp4[:, g:g + 1], scalar2=0.0,
                                        op0=OP.is_gt, op1=OP.add,
                                        accum_out=rank4[:, g:g + 1])
            nc.vector.tensor_scalar(out=mask4[:], in0=rank4[:], scalar1=float(KEEP),
                                    scalar2=None, op0=OP.is_lt)
            nc.vector.tensor_tensor(out=A4m[:], in0=A4[:], in1=mask4[:], op=OP.mult)
            nc.vector.tensor_tensor(out=B4m[:], in0=B4[:], in1=mask4[:], op=OP.mult)

        # =========================== phase 2: write y ===========================
        with ExitStack() as ph2:
            ypool = ph2.enter_context(tc.tile_pool(name="ybuf", bufs=2))
            for g in range(CG):
                y_t = ypool.tile([128, L], f16, tag="yt", name="yt")
                nc.vector.tensor_scalar(out=y_t[:], in0=X_sb[g][:],
                                        scalar1=A4m[:, g:g + 1],
                                        scalar2=B4m[:, g:g + 1],
                                        op0=OP.mult, op1=OP.add)
                eng = nc.sync if g % 2 == 0 else nc.scalar
                eng.dma_start(out=out_d[g * 128:(g + 1) * 128, :], in_=y_t[:])


def _get_nc():
    if "nc" not in _CACHE:
        _CACHE["nc"] = _build_nc()
    return _CACHE["nc"]


def _host_weight_maps(gw1, gb1, gw2, gb2, mw1, mb1, mw2, mb2):
    import ml_dtypes
    f = np.float32
    bf = ml_dtypes.bfloat16
    return {
        "gw1t": np.ascontiguousarray(np.asarray(gw1, f).T.reshape(CG, 128, H).transpose(1, 0, 2).astype(bf)),
        "gb1c": np.ascontiguousarray(np.asarray(gb1, f).reshape(H, 1)),
        "gw2t": np.ascontiguousarray(np.asarray(gw2, f).T.astype(bf)),   # [H,C]
        "gb2c": np.ascontiguousarray(np.asarray(gb2, f).reshape(CG, 128).T),
        "mw1t": np.ascontiguousarray(np.asarray(mw1, f).T.reshape(CG, 128, H).transpose(1, 0, 2).astype(bf)),
        "mb1c": np.ascontiguousarray(np.asarray(mb1, f).reshape(H, 1)),
        "mw2t": np.ascontiguousarray(np.asarray(mw2, f).T),              # [H,2C]
        "mb2c": np.ascontiguousarray(np.asarray(mb2, f).reshape(2 * CG, 128).T),
    }


def _run(inputs, trace=False):
    import ml_dtypes
    from concourse.bass_utils import run_bass_kernel_spmd

    nc = _get_nc()
    x = np.asarray(inputs["x"], np.float32).astype(np.float16)
    c = np.asarray(inputs["c"], np.float32).astype(ml_dtypes.bfloat16)
    wmap = _host_weight_maps(
        inputs["gw1"], inputs["gb1"], inputs["gw2"], inputs["gb2"],
        inputs["mw1"], inputs["mb1"], inputs["mw2"], inputs["mb2"])
    in_maps = [
        dict(wmap, x=np.ascontiguousarray(x[b]), c=np.ascontiguousarray(c[b]))
        for b in range(B)
    ]
    res = run_bass_kernel_spmd(nc, in_maps, core_ids=list(range(B)), trace=trace)
    out = np.stack([res.results[b]["out"] for b in range(B)], axis=0).astype(np.float32)
    return out, res


def kernel(**inputs):
    out, _ = _run(inputs, trace=False)
    return out


# revision 28
# speedup vs baseline: 1.4897x; 1.0747x over previous
"""Trainium2 Bass kernel for the DUAN conditioned-normalization problem (v2).

Contract: kernel(**inputs) takes FULL inputs (B=8 samples), shards one sample
per NeuronCore (8 cores), runs a single Bass/Tile kernel SPMD, and gathers the
full [8, 512, 8192] output.

Per-sample math (matches the jax reference):
  mu_c/var_c over L per channel; mu_l/var_l over (C,L);
  g = sigmoid(gw2 @ relu(gw1 @ c + gb1) + gb2); g_mix = mean_L(g)
  mu = g_mix*mu_c + (1-g_mix)*mu_l ; sigma likewise from sqrt(var+eps)
  gamma,beta = mw2 @ relu(mw1 @ mean_L(c) + mb1) + mb2
  y = (1+gamma)*(x-mu)/sigma + beta
  keep top-k channels by mean_L |y| (k=358), zero the rest.

v2: x staged host-side to fp16 and y written fp16 (upcast on host), halving
the dominant HBM traffic vs fp32 (in 16 MiB, out 8 MiB per core).  The gate
network runs in bf16 on the PE.  Top-k mask on-device via imp ranking.
Measured rel-err ~7e-4 with exact top-k mask agreement.
"""

import sys

sys.path.insert(0, "/opt/trn_rl_repo")

import numpy as np

B = 8
C = 512
L = 8192
H = 128
CG = 4           # channel groups of 128 partitions
SL = 1024        # gate-network super-tile width along L
NST = L // SL    # 8
XC = 4096        # x dma chunk along L
NXC = L // XC    # 2
LB = 512         # bn_stats chunk
NLB = L // LB    # 16
KEEP = max(1, int(C * 0.7))  # 358
EPS = 1e-5

_CACHE = {}


def _build_nc():
    import concourse.bacc as bacc
    import concourse.bass as bass
    import concourse.tile as tile
    from concourse import mybir

    f32 = mybir.dt.float32
    f16 = mybir.dt.float16
    bf16 = mybir.dt.bfloat16
    AF = mybir.ActivationFunctionType
    OP = mybir.AluOpType
    AX = mybir.AxisListType

    nc = bacc.Bacc("TRN2", target_bir_lowering=False, debug=False, num_devices=8)

    x_d = nc.declare_dram_parameter("x", [C, L], f16, isOutput=False)
    c_d = nc.declare_dram_parameter("c", [C, L], bf16, isOutput=False)
    gw1t_d = nc.declare_dram_parameter("gw1t", [128, CG, H], bf16, isOutput=False)
    gb1_d = nc.declare_dram_parameter("gb1c", [H, 1], f32, isOutput=False)
    gw2t_d = nc.declare_dram_parameter("gw2t", [H, C], bf16, isOutput=False)
    gb2_d = nc.declare_dram_parameter("gb2c", [H, CG], f32, isOutput=False)
    mw1t_d = nc.declare_dram_parameter("mw1t", [128, CG, H], bf16, isOutput=False)
    mb1_d = nc.declare_dram_parameter("mb1c", [H, 1], f32, isOutput=False)
    mw2t_d = nc.declare_dram_parameter("mw2t", [H, 2 * C], f32, isOutput=False)
    mb2_d = nc.declare_dram_parameter("mb2c", [H, 2 * CG], f32, isOutput=False)
    out_d = nc.declare_dram_parameter("out", [C, L], f16, isOutput=True)

    with tile.TileContext(nc) as tc:
        _emit(tc, bass, mybir, f32, f16, bf16, AF, OP, AX,
              x_d, c_d, gw1t_d, gb1_d, gw2t_d, gb2_d,
              mw1t_d, mb1_d, mw2t_d, mb2_d, out_d)

    nc.compile()
    return nc


def _emit(tc, bass, mybir, f32, f16, bf16, AF, OP, AX,
          x_d, c_d, gw1t_d, gb1_d, gw2t_d, gb2_d,
          mw1t_d, mb1_d, mw2t_d, mb2_d, out_d):
    from contextlib import ExitStack

    nc = tc.nc

    with ExitStack() as top:
        xpool = top.enter_context(tc.tile_pool(name="xbuf", bufs=1))
        wpool = top.enter_context(tc.tile_pool(name="wts", bufs=1))
        spool = top.enter_context(tc.tile_pool(name="stats", bufs=1))
        scpool = top.enter_context(tc.tile_pool(name="bigscr", bufs=1))
        dpool = top.enter_context(tc.tile_pool(name="dscr", bufs=1, space="DRAM"))
        imp_dr = dpool.tile([128, CG], f32, tag="impdr", name="impdr")

        # ---- weights / biases into SBUF ----
        w1_sb = wpool.tile([128, CG, H], bf16, tag="w1", name="w1")
        nc.sync.dma_start(out=w1_sb[:], in_=gw1t_d[:])
        m1_sb = wpool.tile([128, CG, H], bf16, tag="m1w", name="m1w")
        nc.sync.dma_start(out=m1_sb[:], in_=mw1t_d[:])
        w2_sb = wpool.tile([128, C], bf16, tag="w2", name="w2")
        nc.sync.dma_start(out=w2_sb[:], in_=gw2t_d[:])
        gb1_sb = wpool.tile([128, 1], f32, tag="gb1", name="gb1")
        nc.sync.dma_start(out=gb1_sb[:], in_=gb1_d[:])
        gb2_sb = wpool.tile([128, CG], f32, tag="gb2", name="gb2")
        nc.sync.dma_start(out=gb2_sb[:], in_=gb2_d[:])
        mb1_sb = wpool.tile([128, 1], f32, tag="mb1", name="mb1")
        nc.sync.dma_start(out=mb1_sb[:], in_=mb1_d[:])
        mb2_sb = wpool.tile([128, 2 * CG], f32, tag="mb2", name="mb2")
        nc.sync.dma_start(out=mb2_sb[:], in_=mb2_d[:])
        m2_sb = wpool.tile([128, 2 * C], f32, tag="m2w", name="m2w")
        nc.sync.dma_start(out=m2_sb[:], in_=mw2t_d[:])

        ones_sb = spool.tile([128, 128], f32, tag="ones", name="ones")
        nc.vector.memset(ones_sb[:], 1.0)

        # ---- persistent accumulators ----
        X_sb = [xpool.tile([128, L], f16, tag=f"X{g}", name=f"X{g}") for g in range(CG)]
        stats = [spool.tile([128, NLB, 6], f32, tag=f"bnst{g}", name=f"bnst{g}")
                 for g in range(CG)]
        muvar = spool.tile([128, CG, 2], f32, tag="muvar", name="muvar")
        csum = spool.tile([128, CG * NST], f32, tag="csum", name="csum")
        gacc = spool.tile([128, CG * NST], f32, tag="gacc", name="gacc")
        work = spool.tile([128, 16], f32, tag="work", name="work")
        mu4t = spool.tile([128, CG], f32, tag="mu4t", name="mu4t")
        sg4t = spool.tile([128, CG], f32, tag="sg4t", name="sg4t")
        scal = spool.tile([128, 8], f32, tag="scal", name="scal")
        bl_sb = spool.tile([128, 2], f32, tag="blb", name="blb")
        cs4h = spool.tile([128, CG], bf16, tag="cs4h", name="cs4h")
        hm_sb = spool.tile([128, 1], f32, tag="hm", name="hm")
        A4 = spool.tile([128, CG], f32, tag="A4", name="A4")
        B4 = spool.tile([128, CG], f32, tag="B4", name="B4")
        iacc = spool.tile([128, CG], f32, tag="iacc", name="iacc")
        imp4 = spool.tile([128, CG], f32, tag="imp4", name="imp4")
        T_sb = spool.tile([128, C], f32, tag="Tsb", name="Tsb")
        rank4 = spool.tile([128, CG], f32, tag="rank4", name="rank4")
        mask4 = spool.tile([128, CG], f32, tag="mask4", name="mask4")
        A4m = spool.tile([128, CG], f32, tag="A4m", name="A4m")
        B4m = spool.tile([128, CG], f32, tag="B4m", name="B4m")
        gscr512 = spool.tile([128, C], f32, tag="g512", name="g512")

        # =========================== phase 1 ===========================
        with ExitStack() as ph1:
            cpool = ph1.enter_context(tc.tile_pool(name="cbuf", bufs=2))
            hpool = ph1.enter_context(tc.tile_pool(name="hbuf", bufs=2))
            gspool = ph1.enter_context(tc.tile_pool(name="gscr", bufs=2))
            ps_h = ph1.enter_context(tc.tile_pool(name="psh", bufs=2, space="PSUM"))
            ps_g = ph1.enter_context(tc.tile_pool(name="psg", bufs=2, space="PSUM"))

            cpair = None
            for st in range(NST):
                if st % 2 == 0:
                    p0 = st * SL
                    cpair = [cpool.tile([128, 2 * SL], bf16, tag=f"c{g}",
                                        name=f"c{g}") for g in range(CG)]
                    for g in range(CG):
                        nc.sync.dma_start(
                            out=cpair[g][:],
                            in_=c_d[g * 128:(g + 1) * 128, p0:p0 + 2 * SL],
                        )
                off = (st % 2) * SL
                c_t = [cpair[g][:, off:off + SL] for g in range(CG)]
                # one x chunk per super-tile: [128, XC] for group st//2
                xg, xj = divmod(st, NXC)
                nc.sync.dma_start(
                    out=X_sb[xg][:, xj * XC:(xj + 1) * XC],
                    in_=x_d[xg * 128:(xg + 1) * 128, xj * XC:(xj + 1) * XC],
                )

                # gate layer 1: h = relu(gw1 @ c + gb1), contraction over C
                h_ps = ps_h.tile([128, SL], f32, tag="hps", name="hps")
                for g in range(CG):
                    for q in range(SL // 512):
                        hs = slice(q * 512, (q + 1) * 512)
                        nc.tensor.matmul(h_ps[:, hs], w1_sb[:, g, :],
                                         cpair[g][:, off + q * 512:off + (q + 1) * 512],
                                         start=(g == 0), stop=(g == CG - 1))
                h_sb = hpool.tile([128, SL], bf16, tag="hsb", name="hsb")
                nc.scalar.activation(out=h_sb[:], in_=h_ps[:], func=AF.Relu,
                                     bias=gb1_sb[:], scale=1.0)

                # c channel-sums: 3 groups on DVE, 1 on ACT
                for g in range(CG - 1):
                    nc.vector.reduce_sum(
                        out=csum[:, g * NST + st:g * NST + st + 1],
                        in_=c_t[g], axis=AX.X)
                cscr = gspool.tile([128, SL], bf16, tag="gscr", name="cscr")
                nc.scalar.activation(
                    out=cscr[:], in_=c_t[CG - 1], func=AF.Identity,
                    bias=0.0, scale=1.0,
                    accum_out=csum[:, (CG - 1) * NST + st:(CG - 1) * NST + st + 1])

                # gate layer 2 + sigmoid; mean over L via accum_out
                for g in range(CG):
                    g_ps = ps_g.tile([128, SL], f32, tag="gmps", name="g_ps")
                    for q in range(SL // 512):
                        hs = slice(q * 512, (q + 1) * 512)
                        nc.tensor.matmul(g_ps[:, hs], w2_sb[:, g * 128:(g + 1) * 128],
                                         h_sb[:, hs], start=True, stop=True)
                    g_scr = gspool.tile([128, SL], bf16, tag="gscr", name="g_scr")
                    nc.scalar.activation(out=g_scr[:], in_=g_ps[:], func=AF.Sigmoid,
                                         bias=gb2_sb[:, g:g + 1], scale=1.0,
                                         accum_out=gacc[:, g * NST + st:g * NST + st + 1])

                # x stats for the arrived chunk: bn_stats per 512-wide piece
                for jj in range(XC // LB):
                    j = xj * (XC // LB) + jj
                    nc.vector.bn_stats(out=stats[xg][:, j, :],
                                       in_=X_sb[xg][:, j * LB:(j + 1) * LB])
                if xj == NXC - 1:
                    nc.vector.bn_aggr(out=muvar[:, xg, :], in_=stats[xg][:])

        # =========================== finalize ===========================
        with ExitStack() as fin:
            ps_f = fin.enter_context(tc.tile_pool(name="psf", bufs=1, space="PSUM"))

            # ---- cond MLP first (independent of x stats; PE is idle) ----
            cs4 = work[:, 8:12]
            for g in range(CG):
                nc.vector.reduce_sum(out=cs4[:, g:g + 1],
                                     in_=csum[:, g * NST:(g + 1) * NST], axis=AX.X)
            nc.vector.tensor_copy(out=cs4h[:], in_=cs4)
            hm_ps = ps_f.tile([128, 1], f32, tag="hmps", name="hmps")
            for g in range(CG):
                nc.tensor.matmul(hm_ps[:], m1_sb[:, g, :], cs4h[:, g:g + 1],
                                 start=(g == 0), stop=(g == CG - 1))
            nc.scalar.activation(out=hm_sb[:], in_=hm_ps[:], func=AF.Relu,
                                 bias=mb1_sb[:], scale=1.0 / L)
            gb_ps = ps_f.tile([128, 2 * CG], f32, tag="gbps", name="gbps")
            for o in range(2 * CG):
                nc.tensor.matmul(gb_ps[:, o:o + 1],
                                 m2_sb[:, o * 128:(o + 1) * 128], hm_sb[:],
                                 start=True, stop=True)

            # ---- channel stats (bn_aggr already done per-group in phase 1) ----
            mu_c = muvar[:, :, 0]   # [128, CG] strided
            var_c = muvar[:, :, 1]

            # rowstats: cols 0:4 = mu_c, 4:8 = E[x^2] = var_c + mu_c^2
            nc.vector.tensor_copy(out=work[:, 0:4], in_=mu_c)
            nc.vector.tensor_tensor(out=work[:, 4:8], in0=mu_c, in1=mu_c, op=OP.mult)
            nc.vector.tensor_add(out=work[:, 4:8], in0=work[:, 4:8], in1=var_c)

            # cross-partition sums -> [1, 8]
            colsum = ps_f.tile([128, 8], f32, tag="colsum", name="colsum")
            nc.tensor.matmul(colsum[0:1, :], ones_sb[:, 0:1], work[:, 0:8],
                             start=True, stop=True)

            # partition-0 scalar math: mu_l, var_l, sigma_l
            nc.vector.reduce_sum(out=scal[0:1, 0:1], in_=colsum[0:1, 0:4], axis=AX.X)
            nc.vector.tensor_scalar(out=scal[0:1, 0:1], in0=scal[0:1, 0:1],
                                    scalar1=1.0 / C, scalar2=None, op0=OP.mult)
            nc.vector.reduce_sum(out=scal[0:1, 2:3], in_=colsum[0:1, 4:8], axis=AX.X)
            nc.vector.tensor_scalar(out=scal[0:1, 2:3], in0=scal[0:1, 2:3],
                                    scalar1=1.0 / C, scalar2=None, op0=OP.mult)
            nc.vector.tensor_tensor(out=scal[0:1, 3:4], in0=scal[0:1, 0:1],
                                    in1=scal[0:1, 0:1], op=OP.mult)
            nc.vector.tensor_tensor(out=scal[0:1, 1:2], in0=scal[0:1, 2:3],
                                    in1=scal[0:1, 3:4], op=OP.subtract)
            # sigma_l = sqrt(var_l + eps), one Newton polish
            nc.vector.tensor_scalar(out=scal[0:1, 4:5], in0=scal[0:1, 1:2],
                                    scalar1=EPS, scalar2=None, op0=OP.add)
            nc.scalar.activation(out=scal[0:1, 1:2], in_=scal[0:1, 4:5],
                                 func=AF.Sqrt, bias=0.0, scale=1.0)
            nc.vector.reciprocal(out=scal[0:1, 5:6], in_=scal[0:1, 1:2])
            nc.vector.tensor_tensor(out=scal[0:1, 5:6], in0=scal[0:1, 4:5],
                                    in1=scal[0:1, 5:6], op=OP.mult)
            nc.vector.tensor_add(out=scal[0:1, 1:2], in0=scal[0:1, 1:2],
                                 in1=scal[0:1, 5:6])
            nc.vector.tensor_scalar(out=scal[0:1, 1:2], in0=scal[0:1, 1:2],
                                    scalar1=0.5, scalar2=None, op0=OP.mult)

            # broadcast (mu_l, sigma_l) to all partitions
            bl_ps = ps_f.tile([128, 2], f32, tag="blps", name="blps")
            nc.tensor.matmul(bl_ps[:], ones_sb[0:1, :], scal[0:1, 0:2],
                             start=True, stop=True)
            nc.vector.tensor_copy(out=bl_sb[:], in_=bl_ps[:])
            mu_l = bl_sb[:, 0:1]
            sig_l = bl_sb[:, 1:2]

            # sigma_c = sqrt(var_c + eps) + Newton polish
            vpe4 = work[:, 12:16]
            nc.vector.tensor_scalar(out=vpe4, in0=var_c, scalar1=EPS,
                                    scalar2=None, op0=OP.add)
            sig4 = work[:, 8:12]
            nc.scalar.activation(out=sig4, in_=vpe4, func=AF.Sqrt,
                                 bias=0.0, scale=1.0)
            t4s = work[:, 4:8]
            nc.vector.reciprocal(out=t4s, in_=sig4)
            nc.vector.tensor_tensor(out=t4s, in0=vpe4, in1=t4s, op=OP.mult)
            nc.vector.tensor_add(out=sig4, in0=sig4, in1=t4s)
            nc.vector.tensor_scalar(out=sig4, in0=sig4, scalar1=0.5,
                                    scalar2=None, op0=OP.mult)

            # g_mix = sum(gacc) / L
            gm4 = work[:, 12:16]
            for g in range(CG):
                nc.vector.reduce_sum(out=gm4[:, g:g + 1],
                                     in_=gacc[:, g * NST:(g + 1) * NST], axis=AX.X)
            nc.vector.tensor_scalar(out=gm4, in0=gm4, scalar1=1.0 / L,
                                    scalar2=None, op0=OP.mult)

            # mu = mu_l + g_mix*(mu_c - mu_l); sigma likewise
            nc.vector.tensor_scalar(out=mu4t[:], in0=work[:, 0:4], scalar1=mu_l,
                                    scalar2=None, op0=OP.subtract)
            nc.vector.tensor_tensor(out=mu4t[:], in0=mu4t[:], in1=gm4, op=OP.mult)
            nc.vector.tensor_scalar(out=mu4t[:], in0=mu4t[:], scalar1=mu_l,
                                    scalar2=None, op0=OP.add)
            nc.vector.tensor_scalar(out=sg4t[:], in0=sig4, scalar1=sig_l,
                                    scalar2=None, op0=OP.subtract)
            nc.vector.tensor_tensor(out=sg4t[:], in0=sg4t[:], in1=gm4, op=OP.mult)
            nc.vector.tensor_scalar(out=sg4t[:], in0=sg4t[:], scalar1=sig_l,
                                    scalar2=None, op0=OP.add)

            # A = (1+gamma)/sigma ; B = beta - mu*A
            inv4 = work[:, 0:4]
            nc.vector.reciprocal(out=inv4, in_=sg4t[:])
            gam4 = work[:, 4:8]
            nc.vector.tensor_add(out=gam4, in0=gb_ps[:, 0:CG], in1=mb2_sb[:, 0:CG])
            nc.vector.tensor_scalar(out=gam4, in0=gam4, scalar1=1.0,
                                    scalar2=None, op0=OP.add)
            bet4 = work[:, 8:12]
            nc.vector.tensor_add(out=bet4, in0=gb_ps[:, CG:2 * CG],
                                 in1=mb2_sb[:, CG:2 * CG])
            nc.vector.tensor_tensor(out=A4[:], in0=gam4, in1=inv4, op=OP.mult)
            muA = work[:, 12:16]
            nc.vector.tensor_tensor(out=muA, in0=mu4t[:], in1=A4[:], op=OP.mult)
            nc.vector.tensor_tensor(out=B4[:], in0=bet4, in1=muA, op=OP.subtract)

            # ---- imp_g = sum_L |A x + B|: ACT for groups 0-2, DVE for 3 ----
            for g in range(CG):
                if g < 3:
                    iscr = scpool.tile([128, L], bf16, tag="bigscr", name="iscr")
                    nc.scalar.activation(out=iscr[:], in_=X_sb[g][:], func=AF.Abs,
                                         bias=B4[:, g:g + 1], scale=A4[:, g:g + 1],
                                         accum_out=iacc[:, g:g + 1])
                else:
                    dscr = scpool.tile([128, L], f32, tag="bigscr32", name="dscr")
                    nc.vector.tensor_scalar(out=dscr[:], in0=X_sb[g][:],
                                            scalar1=A4[:, g:g + 1],
                                            scalar2=B4[:, g:g + 1],
                                            op0=OP.mult, op1=OP.add)
                    nc.vector.tensor_reduce(out=iacc[:, g:g + 1], in_=dscr[:],
                                            axis=AX.X, op=OP.add,
                                            apply_absolute_value=True)
            nc.vector.tensor_copy(out=imp4[:], in_=iacc[:])

            # ---- top-k mask: rank[i] = #{j: imp[j] > imp[i]} ----
            nc.scalar.dma_start(out=imp_dr[:], in_=imp4[:])
            imp_flat = imp_dr[:]
            bcast = bass.AP(tensor=imp_flat.tensor, offset=imp_flat.offset,
                            ap=[[0, 128], [1, C]])
            nc.scalar.dma_start(out=T_sb[:], in_=bcast)
            for g in range(CG):
                nc.vector.tensor_scalar(out=gscr512[:], in0=T_sb[:],
                                        scalar1=imp4[:, g:g + 1], scalar2=0.0,
                                        op0=OP.is_gt, op1=OP.add,
                                        accum_out=rank4[:, g:g + 1])
            nc.vector.tensor_scalar(out=mask4[:], in0=rank4[:], scalar1=float(KEEP),
                                    scalar2=None, op0=OP.is_lt)
            nc.vector.tensor_tensor(out=A4m[:], in0=A4[:], in1=mask4[:], op=OP.mult)
            nc.vector.tensor_tensor(out=B4m[:], in0=B4[:], in1=mask4[:], op=OP.mult)

        # =========================== phase 2: write y ===========================
        with ExitStack() as ph2:
            ypool = ph2.enter_context(tc.tile_pool(name="ybuf", bufs=4))
            for g in range(CG):
                y_t = ypool.tile([128, L], f16, tag="yt", name="yt")
                nc.vector.tensor_scalar(out=y_t[:], in0=X_sb[g][:],
                                        scalar1=A4m[:, g:g + 1],
                                        scalar2=B4m[:, g:g + 1],
                                        op0=OP.mult, op1=OP.add)
                eng = nc.sync if g % 2 == 0 else nc.scalar
                eng.dma_start(out=out_d[g * 128:(g + 1) * 128, :], in_=y_t[:])


def _get_nc():
    if "nc" not in _CACHE:
        _CACHE["nc"] = _build_nc()
    return _CACHE["nc"]


def _host_weight_maps(gw1, gb1, gw2, gb2, mw1, mb1, mw2, mb2):
    import ml_dtypes
    f = np.float32
    bf = ml_dtypes.bfloat16
    return {
        "gw1t": np.ascontiguousarray(np.asarray(gw1, f).T.reshape(CG, 128, H).transpose(1, 0, 2).astype(bf)),
        "gb1c": np.ascontiguousarray(np.asarray(gb1, f).reshape(H, 1)),
        "gw2t": np.ascontiguousarray(np.asarray(gw2, f).T.astype(bf)),   # [H,C]
        "gb2c": np.ascontiguousarray(np.asarray(gb2, f).reshape(CG, 128).T),
        "mw1t": np.ascontiguousarray(np.asarray(mw1, f).T.reshape(CG, 128, H).transpose(1, 0, 2).astype(bf)),
        "mb1c": np.ascontiguousarray(np.asarray(mb1, f).reshape(H, 1)),
        "mw2t": np.ascontiguousarray(np.asarray(mw2, f).T),              # [H,2C]
        "mb2c": np.ascontiguousarray(np.asarray(mb2, f).reshape(2 * CG, 128).T),
    }


def _run(inputs, trace=False):
    import ml_dtypes
    from concourse.bass_utils import run_bass_kernel_spmd

    nc = _get_nc()
    x = np.asarray(inputs["x"], np.float32).astype(np.float16)
    c = np.asarray(inputs["c"], np.float32).astype(ml_dtypes.bfloat16)
    wmap = _host_weight_maps(
        inputs["gw1"], inputs["gb1"], inputs["gw2"], inputs["gb2"],
        inputs["mw1"], inputs["mb1"], inputs["mw2"], inputs["mb2"])
    in_maps = [
        dict(wmap, x=np.ascontiguousarray(x[b]), c=np.ascontiguousarray(c[b]))
        for b in range(B)
    ]
    res = run_bass_kernel_spmd(nc, in_maps, core_ids=list(range(B)), trace=trace)
    out = np.stack([res.results[b]["out"] for b in range(B)], axis=0).astype(np.float32)
    return out, res


def kernel(**inputs):
    out, _ = _run(inputs, trace=False)
    return out
